# revision 14
# baseline (speedup 1.0000x reference)
"""MoE gate kernel for Trainium2 (8 NeuronCores, SPMD).

Computes, for hidden_states [4, 4096, 4096] f32 and gate_weight [8, 4096] f32:
    logits = hidden @ gate_weight.T          # [tokens, 8]
    p      = softmax(logits)                 # [tokens, 8]
    topk_w, topk_i = top_k(p, 2); topk_w /= topk_w.sum(-1, keepdims=True)

Sharding: data-parallel over tokens (B*S = 16384 -> 2048 tokens/core), gate
weight replicated.  Per core the kernel streams X tiles of 128 tokens with a
strided DMA that puts h_hi = h//32 on the 128 SBUF partitions (DRAM runs of
128B), so the PE can contract over h with 32 accumulating matmuls per tile
(lhsT = X chunk [128h x 128t], rhs = W chunk [128h x 8e]).  Top-2 + renorm
uses the DVE max/max_index sort instructions; the renormalized weights reduce
to w1 = 1/(1+exp(m2-m1)), w2 = 1-w1 (softmax denominator cancels).
"""

import numpy as np

H = 4096            # hidden size
E = 8               # experts
P = 128             # SBUF partitions
F = H // P          # 32 f32 per partition per token
T_TILE = 128        # tokens per tile (PSUM partition dim)
N_CORES = 8
TOKENS_TOTAL = 4 * 4096
TOKENS_PER_CORE = TOKENS_TOTAL // N_CORES   # 2048
N_TILES = TOKENS_PER_CORE // T_TILE         # 16


def _emit_body(nc, mybir, pools, w_sb, x_r, wq, iq, n_tiles):
    xpool, cpool, spool, psum_pool = pools
    sorted_w = cpool.tile([P, n_tiles, E], mybir.dt.float32)
    idx_w = cpool.tile([P, n_tiles, E], mybir.dt.uint32)

    # Walrus allows at most ONE sync wait on a PE Matmult (fp32 S3_LW), so
    # the loop is structured to never need more: 8 token-tiles' logits
    # accumulate into one PSUM bank ([128, 8tile, 8expert]); a tiny
    # "bank-claim" matmul reading only the long-resident w_sb absorbs the
    # bank-release wait; each tile's first matmul then carries only its
    # x-DMA wait, and the group's logits leave PSUM in one batched copy.
    group = min(8, n_tiles)
    assert n_tiles % group == 0
    for g in range(n_tiles // group):
        psg = psum_pool.tile([T_TILE, group, E], mybir.dt.float32)
        nc.tensor.matmul(
            psg[0:1, 0:1, 0], w_sb[:, 0:1, 0], w_sb[:, 0:1, 0],
            start=True, stop=True, skip_group_check=True,
        )
        for il in range(group):
            i = g * group + il
            xt = xpool.tile([P, T_TILE, F], mybir.dt.float32)
            nc.sync.dma_start(xt[:], x_r[i])
            for j in range(F):
                nc.tensor.matmul(
                    psg[:, il],
                    xt[:, :, j],
                    w_sb[:, :, j],
                    start=(j == 0),
                    stop=(j == F - 1),
                )
        logits = spool.tile([T_TILE, group, E], mybir.dt.float32)
        nc.vector.tensor_copy(logits[:], psg[:])
        for il in range(group):
            i = g * group + il
            nc.vector.max(out=sorted_w[:, i], in_=logits[:, il])
            nc.vector.max_index(
                out=idx_w[:, i], in_max=sorted_w[:, i], in_values=logits[:, il]
            )

    # Batched renormalization over all tiles: w1 = 1/(1+e^(m2-m1)),
    # w2 = e^(m2-m1)/(1+e^(m2-m1)).  The full-softmax denominator cancels
    # in the reference's top-k renormalization, so only the top-2 logit
    # gap matters.
    m1 = sorted_w[:, :, 0]
    m2 = sorted_w[:, :, 1]
    d = cpool.tile([P, n_tiles], mybir.dt.float32)
    nc.vector.tensor_sub(d[:], m2, m1)
    t = cpool.tile([P, n_tiles], mybir.dt.float32)
    nc.scalar.activation(t[:], d[:], mybir.ActivationFunctionType.Exp)
    denom = cpool.tile([P, n_tiles], mybir.dt.float32)
    nc.vector.tensor_scalar_add(denom[:], t[:], 1.0)
    r = cpool.tile([P, n_tiles], mybir.dt.float32)
    nc.vector.reciprocal(r[:], denom[:])

    wout = cpool.tile([P, n_tiles, 2], mybir.dt.float32)
    nc.vector.tensor_copy(wout[:, :, 0], r[:])
    nc.vector.tensor_mul(wout[:, :, 1], t[:], r[:])
    iout = cpool.tile([P, n_tiles, 2], mybir.dt.uint32)
    nc.vector.tensor_copy(iout[:], idx_w[:, :, 0:2])

    nc.gpsimd.dma_start(wq[:], wout[:])
    nc.gpsimd.dma_start(iq[:], iout[:])


def _legalize_sync_waits(nc, mybir):
    """Split surplus sync waits onto EventSemaphore prefix instructions.

    Walrus's TPB instruction structs have a single `events` wait slot; PE
    Matmult and the HWDGE DMA DIRECT2D form reject instructions with >1
    sync wait.  The same engine sequencer executes an EventSemaphore
    (CTRL_ES) wait-only instruction in program order, so hoisting all but
    one wait onto ES prefixes is semantics-preserving.
    """
    limit = 1
    n = 0
    for bb in nc.main_func.blocks:
        out, changed = [], False
        for ins in bb.instructions:
            si = ins.sync_info
            if si is not None and len(si.on_wait) > limit:
                waits = list(si.on_wait)
                for w in waits[:-limit]:
                    es = mybir.InstEventSemaphore(
                        name=f"ESleg-{n}", engine=ins.engine, ins=[], outs=[],
                        sync_info=mybir.SyncInfo(on_wait=[w], on_update=[]),
                    )
                    out.append(es)
                    n += 1
                ins.sync_info = mybir.SyncInfo(
                    on_wait=waits[-limit:], on_update=list(si.on_update)
                )
                changed = True
            out.append(ins)
        if changed:
            bb.instructions = out
    return n


def build_program(tokens_per_core: int = TOKENS_PER_CORE, reps: int = 1,
                  legalize: bool = True):
    import concourse.bass as bass
    import concourse.mybir as mybir
    from concourse.tile import TileContext

    n_tiles = tokens_per_core // T_TILE
    nc = bass.Bass("TRN2", debug=False)
    x = nc.declare_dram_parameter(
        "x", [tokens_per_core, H], mybir.dt.float32, isOutput=False
    )
    w = nc.declare_dram_parameter("w", [E, H], mybir.dt.float32, isOutput=False)
    wq = nc.declare_dram_parameter(
        "wq", [P, n_tiles, 2], mybir.dt.float32, isOutput=True
    )
    iq = nc.declare_dram_parameter(
        "iq", [P, n_tiles, 2], mybir.dt.uint32, isOutput=True
    )

    # (p, t, f): h = p*F + f; DRAM-side runs of F contiguous f32 (128B)
    x_r = x[:].rearrange("(n t) (p f) -> n p t f", t=T_TILE, p=P)
    # (p, e, j): W^T chunk j is w_sb[:, :, j] = W[e, p*F + j]
    w_r = w[:].rearrange("e (p j) -> p e j", p=P)

    with TileContext(nc) as tc:
        with (
            tc.tile_pool(name="xpool", bufs=4) as xpool,
            tc.tile_pool(name="cpool", bufs=1) as cpool,
            tc.tile_pool(name="spool", bufs=4) as spool,
            tc.tile_pool(name="psum", bufs=8, space="PSUM") as psum_pool,
        ):
            w_sb = cpool.tile([P, E, F], mybir.dt.float32)
            nc.sync.dma_start(w_sb[:], w_r)
            for _rep in range(reps):
                _emit_body(
                    nc, mybir, (xpool, cpool, spool, psum_pool),
                    w_sb, x_r, wq, iq, n_tiles,
                )
    if legalize:
        _legalize_sync_waits(nc, mybir)
    return nc


def shard_inputs(hidden_states, gate_weight):
    hs = np.ascontiguousarray(np.asarray(hidden_states, dtype=np.float32)).reshape(
        TOKENS_TOTAL, H
    )
    gw = np.ascontiguousarray(np.asarray(gate_weight, dtype=np.float32))
    return [
        {"x": hs[c * TOKENS_PER_CORE : (c + 1) * TOKENS_PER_CORE], "w": gw}
        for c in range(N_CORES)
    ]


def assemble(results):
    ws, idxs = [], []
    for c in range(N_CORES):
        wq = np.asarray(results[c]["wq"]).reshape(P, N_TILES, 2)
        iq = np.asarray(results[c]["iq"]).reshape(P, N_TILES, 2)
        # token (core-local) = tile*128 + p
        ws.append(np.transpose(wq, (1, 0, 2)).reshape(TOKENS_PER_CORE, 2))
        idxs.append(np.transpose(iq, (1, 0, 2)).reshape(TOKENS_PER_CORE, 2))
    w_full = np.concatenate(ws, 0).reshape(4, 4096, 2).astype(np.float32)
    i_full = np.concatenate(idxs, 0).reshape(4, 4096, 2).astype(np.int32)
    return w_full, i_full


def kernel(hidden_states, gate_weight):
    from concourse.bass_utils import run_bass_kernel_spmd

    nc = build_program()
    in_maps = shard_inputs(hidden_states, gate_weight)
    br = run_bass_kernel_spmd(nc, in_maps, list(range(N_CORES)), trace=False)
    return assemble(br.results)
